# revision 18
# baseline (speedup 1.0000x reference)
"""Trainium2 Bass kernel for nn_Metric_35545149342437 (RelationNet-style few-shot metric).

Sharding: data-parallel over the 8 episodes (one per NeuronCore). Conv-stack
BatchNorm uses batch statistics over ALL episodes' images; per-layer channel
sum/sumsq partials are AllReduced across the 8 cores, split into a support-
group and a query-group collective per layer so each one's latency hides
behind compute (support stats complete early — after pair 2 — and query-group
results are only needed from pair 2 of the next conv layer onward).

All matmuls run in bf16 (fp32 PSUM accumulation); elementwise work is spread
across scalar/vector/gpsimd engines. conv1 uses a 27-tap im2col (K=54 with
two images packed) so each output chunk is a single matmul.
"""
import numpy as np
import ml_dtypes

import concourse.bacc as bacc
import concourse.bass as bass
import concourse.mybir as mybir
from concourse import tile
from concourse.bass_utils import run_bass_kernel_spmd

F32 = mybir.dt.float32
F32R = mybir.dt.float32r
BF16 = mybir.dt.bfloat16
AF = mybir.ActivationFunctionType
ALU = mybir.AluOpType
AX = mybir.AxisListType

B, N_WAY, Q, IMG = 8, 5, 15, 84
NIMG = N_WAY + Q          # 20 images per episode/core
NPAIR = NIMG // 2         # 10 pairs; pair p = images (2p, 2p+1)
CF = 64
EPS = 1e-5
IMG2 = IMG * IMG          # 7056
IMGSTRIDE = 3 * IMG2 + 8  # padded per-image stride (elements) for im2col tail reads

PIX = {1: 41 * 41, 2: 19 * 19, 3: 17 * 17, 4: 15 * 15}
CNT_S = {l: 40 * PIX[l] for l in PIX}
CNT_Q = {l: 120 * PIX[l] for l in PIX}

NPBF = ml_dtypes.bfloat16


# ---------------------------------------------------------------- host packing
def _pack_weights(inp):
    out = {}
    cw1 = np.asarray(inp["cw1"], np.float32)  # (64,3,3,3) (O,C,KH,KW)
    # w1t (64,128): row p=c*9+di*3+dj -> cols 0:64 (img A); row 27+p -> cols 64:128
    w1t = np.zeros((64, 128), np.float32)
    for di in range(3):
        for dj in range(3):
            for c in range(3):
                p = c * 9 + di * 3 + dj
                w1t[p, 0:64] = cw1[:, c, di, dj]
                w1t[27 + p, 64:128] = cw1[:, c, di, dj]
    out["w1t"] = w1t.astype(NPBF)

    for l, name in [(2, "cw2"), (3, "cw3"), (4, "cw4")]:
        cw = np.asarray(inp[name], np.float32)  # (64,64,3,3)
        blk = np.zeros((128, 9 * 128), np.float32)
        for di in range(3):
            for dj in range(3):
                t = di * 3 + dj
                wt = cw[:, :, di, dj].T  # (C_in, O)
                blk[0:64, t * 128 : t * 128 + 64] = wt
                blk[64:128, t * 128 + 64 : t * 128 + 128] = wt
        out[f"w{l}blk"] = blk.astype(NPBF)

    # bn params stacked [g;g],[b;b]: (128, 8) col l*2 = g_l, l*2+1 = b_l
    bnp = np.zeros((128, 8), np.float32)
    for i, l in enumerate([1, 2, 3, 4]):
        g = np.asarray(inp[f"bg{l}"], np.float32)
        b = np.asarray(inp[f"bb{l}"], np.float32)
        bnp[0:64, i * 2] = g
        bnp[64:128, i * 2] = g
        bnp[0:64, i * 2 + 1] = b
        bnp[64:128, i * 2 + 1] = b
    out["bnp"] = bnp

    # inverse-count tiles for BN mean/var, per layer, combos [s|s, s|q, q|q]: (128, 12)
    invc = np.zeros((128, 12), np.float32)
    for i, l in enumerate([1, 2, 3, 4]):
        cs, cq = 1.0 / CNT_S[l], 1.0 / CNT_Q[l]
        invc[0:64, i * 3 + 0] = cs
        invc[64:128, i * 3 + 0] = cs
        invc[0:64, i * 3 + 1] = cs
        invc[64:128, i * 3 + 1] = cq
        invc[0:64, i * 3 + 2] = cq
        invc[64:128, i * 3 + 2] = cq
    out["invc"] = invc

    # g-MLP layer 1 split: gW1 (132,256): rows 0:66 = Ws (support), 66:132 = Wq
    gW1 = np.asarray(inp["gW1"], np.float32)
    gb1 = np.asarray(inp["gb1"], np.float32)
    ii, jj = np.meshgrid(np.arange(3), np.arange(3), indexing="ij")
    coord = (np.stack([ii, jj]).astype(np.float32) / 3.0).reshape(2, 9)  # (2,9)
    out["gwsA"] = (gW1[0:64] / 25.0).astype(NPBF)      # (64,256) stationary K=64
    out["gwqB"] = (gW1[66:130] / 25.0).astype(NPBF)    # (64,256)
    cA = coord.T @ gW1[64:66]                           # (9,256)
    cB = coord.T @ gW1[130:132]                         # (9,256)
    abase = np.zeros((128, 18), np.float32)             # col mh*9+p: cA[p, mh*128+row] + gb1
    qbase = np.zeros((128, 18), np.float32)
    for mh in range(2):
        abase[:, mh * 9 : mh * 9 + 9] = (cA[:, mh * 128 : mh * 128 + 128] + gb1[mh * 128 : mh * 128 + 128]).T
        qbase[:, mh * 9 : mh * 9 + 9] = cB[:, mh * 128 : mh * 128 + 128].T
    out["abase"] = abase
    out["qbase"] = qbase

    # gW2/3/4, fW1/2: (128, 512): col kh*256 + m
    for name in ["gW2", "gW3", "gW4", "fW1", "fW2"]:
        W = np.asarray(inp[name], np.float32)  # (256,256)
        t = np.zeros((128, 512), np.float32)
        t[:, 0:256] = W[0:128]
        t[:, 256:512] = W[128:256]
        out[name.lower() + "t"] = t.astype(NPBF)
    for name in ["gb2", "gb3", "gb4", "fb1", "fb2"]:
        v = np.asarray(inp[name], np.float32)
        t = np.zeros((128, 2), np.float32)
        t[:, 0] = v[0:128]
        t[:, 1] = v[128:256]
        out[name.lower() + "t"] = t
    fW3 = np.asarray(inp["fW3"], np.float32)  # (256,128)
    t = np.zeros((128, 256), np.float32)
    t[:, 0:128] = fW3[0:128]
    t[:, 128:256] = fW3[128:256]
    out["fw3t"] = t.astype(NPBF)
    fb3 = np.asarray(inp["fb3"], np.float32)
    out["fb3t"] = fb3.reshape(128, 1).copy()
    out["fw4t"] = np.asarray(inp["fW4"], np.float32).astype(NPBF)  # (128,64)
    misc = np.zeros((128, 2), np.float32)
    misc[0:64, 0] = np.asarray(inp["fb4"], np.float32)
    misc[0:64, 1] = 1.0
    out["miscb"] = misc
    out["zbf"] = np.zeros((128, 512), NPBF)
    return out


def _per_core_inputs(inp, b):
    sx = np.asarray(inp["support_x"], np.float32)[b]  # (5,3,84,84)
    qx = np.asarray(inp["query_x"], np.float32)[b]    # (15,3,84,84)
    raw = np.concatenate([sx, qx], 0).reshape(NIMG, 3 * IMG2)
    imgs = np.zeros((NIMG, IMGSTRIDE), NPBF)
    imgs[:, : 3 * IMG2] = raw.astype(NPBF)
    return {"imgs": imgs}


# ---------------------------------------------------------------- kernel build
def _apv(base, off, dims):
    """View into base AP: copy partition dim, add free dims, extra element offset."""
    return bass.AP(tensor=base.tensor, offset=base.offset + off,
                   ap=[list(base.ap[0])] + [list(d) for d in dims])


def build_kernel(debug=False):
    nc = bacc.Bacc("TRN2", target_bir_lowering=False, debug=False, num_devices=8)

    def dram_in(name, shape, dt=F32):
        return nc.dram_tensor(name, list(shape), dt, kind="ExternalInput")

    imgs = dram_in("imgs", (NIMG, IMGSTRIDE), BF16)
    w1blk = dram_in("w1t", (64, 128), BF16)
    w2blk = dram_in("w2blk", (128, 9 * 128), BF16)
    w3blk = dram_in("w3blk", (128, 9 * 128), BF16)
    w4blk = dram_in("w4blk", (128, 9 * 128), BF16)
    bnp = dram_in("bnp", (128, 8))
    invc = dram_in("invc", (128, 12))
    gwsA = dram_in("gwsA", (64, 256), BF16)
    gwqB = dram_in("gwqB", (64, 256), BF16)
    abase = dram_in("abase", (128, 18))
    qbase = dram_in("qbase", (128, 18))
    gw2t = dram_in("gw2t", (128, 512), BF16)
    gw3t = dram_in("gw3t", (128, 512), BF16)
    gw4t = dram_in("gw4t", (128, 512), BF16)
    gb2t = dram_in("gb2t", (128, 2))
    gb3t = dram_in("gb3t", (128, 2))
    gb4t = dram_in("gb4t", (128, 2))
    fw1t = dram_in("fw1t", (128, 512), BF16)
    fw2t = dram_in("fw2t", (128, 512), BF16)
    fb1t = dram_in("fb1t", (128, 2))
    fb2t = dram_in("fb2t", (128, 2))
    fw3t = dram_in("fw3t", (128, 256), BF16)
    fb3t = dram_in("fb3t", (128, 1))
    fw4t = dram_in("fw4t", (128, 64), BF16)
    miscb = dram_in("miscb", (128, 2))
    zbf = dram_in("zbf", (128, 512), BF16)

    loss_out = nc.dram_tensor("loss", [1, 75], F32, kind="ExternalOutput")
    dbg = {}
    if debug:
        for name, shape, dt in [
            ("d_p1n", (128, NPAIR * 1681), BF16),
            ("d_p2n", (128, NPAIR * 361), BF16),
            ("d_l3n", (128, NPAIR * 323), BF16),
            ("d_l4n", (128, NPAIR * 285), BF16),
            ("d_f64", (64, 184), BF16),
            ("d_xf", (128, 152), F32),
            ("d_G", (64, 16), F32),
        ]:
            dbg[name] = nc.dram_tensor(name, list(shape), dt, kind="ExternalOutput")

    with tile.TileContext(nc) as tc:
        with (
            tc.tile_pool(name="psum", bufs=4, space="PSUM") as psum,
            tc.tile_pool(name="dram", bufs=16, space="DRAM") as dram,
            tc.tile_pool(name="persist", bufs=1) as pp,
        ):
            cpool_cm = tc.tile_pool(name="convp", bufs=1)
            cp = cpool_cm.__enter__()
            w1t = cp.tile([64, 128], BF16)
            nc.scalar.dma_start(w1t[:], w1blk[:])
            bnpt = cp.tile([128, 8], F32)
            nc.scalar.dma_start(bnpt[:], bnp[:])
            invct = cp.tile([128, 12], F32)
            nc.scalar.dma_start(invct[:], invc[:])
            wblk = {}
            for l, wsrc in [(2, w2blk), (3, w3blk), (4, w4blk)]:
                wblk[l] = cp.tile([128, 9 * 128], BF16, tag=f"wblk{l}", name=f"wblk{l}")
                nc.scalar.dma_start(wblk[l][:], wsrc[:])

            # persistent activations (bf16)
            p1 = [cp.tile([128, 1772], BF16, tag=f"p1_{p}", name=f"p1_{p}") for p in range(NPAIR)]
            p2 = [cp.tile([128, 368], BF16, tag=f"p2_{p}", name=f"p2_{p}") for p in range(NPAIR)]
            l3 = [cp.tile([128, 328], BF16, tag=f"l3_{p}", name=f"l3_{p}") for p in range(NPAIR)]
            l4 = [cp.tile([128, 288], BF16, tag=f"l4_{p}", name=f"l4_{p}") for p in range(NPAIR)]
            feats = pp.tile([64, 184], BF16)
            nc.sync.dma_start(feats[:, 180:184], zbf[:][0:64, 0:4])
            for _p in range(NPAIR):
                nc.sync.dma_start(p1[_p][:, 1681:1772], zbf[:][:, 0:91])
                nc.sync.dma_start(p2[_p][:, 361:368], zbf[:][:, 0:7])

            # per-layer stats (col 2p = sum, 2p+1 = sumsq; partition halves = img parity)
            stats = {l: cp.tile([128, 20], F32, tag=f"st{l}", name=f"st{l}") for l in [1, 2, 3, 4]}
            stats2 = {l: cp.tile([64, 20], F32, tag=f"st2{l}", name=f"st2{l}") for l in [1, 2, 3, 4]}
            sq_scr = cp.tile([128, 1681], BF16, tag="sq_scr")
            bout = {}   # (layer, grp) -> dram AllReduce output tile

            # ---------------- BN helpers ----------------
            def launch_group_allreduce(layer, grp):
                """grp 0 = support (pairs 0-2 even / 0-1 odd), 1 = query."""
                st, st2 = stats[layer], stats2[layer]
                if grp == 0:
                    nc.sync.dma_start(st2[:, 0:4], st[64:128, 0:4])
                    combos = [(0, 3, 0, 2), (1, 3, 1, 2)]
                else:
                    nc.sync.dma_start(st2[:, 4:20], st[64:128, 4:20])
                    combos = [(6, 7, 4, 8), (7, 7, 5, 8)]
                G = cp.tile([64, 2], F32, tag="G", name="G")
                tmp = cp.tile([64, 2], F32, tag="Gtmp", name="Gtmp")
                for k, (off_e, n_e, off_o, n_o) in enumerate(combos):
                    ev = st[0:64, off_e : off_e + 2 * (n_e - 1) + 1 : 2]
                    ov = st2[0:64, off_o : off_o + 2 * (n_o - 1) + 1 : 2]
                    nc.vector.tensor_reduce(tmp[:, k : k + 1], ev, axis=AX.X, op=ALU.add)
                    nc.vector.tensor_reduce(G[:, k : k + 1], ov, axis=AX.X, op=ALU.add)
                nc.vector.tensor_tensor(G[:], G[:], tmp[:], ALU.add)
                bin_ = dram.tile([64, 2], F32, name="arin")
                bo = dram.tile([64, 2], F32, name="arout")
                nc.sync.dma_start(bin_[:], G[:])
                nc.gpsimd.collective_compute(
                    "AllReduce", ALU.add, ins=[bin_.opt()], outs=[bo.opt()],
                    replica_groups=[list(range(8))],
                )
                bout[(layer, grp)] = bo
                if debug:
                    c0 = (layer - 1) * 4 + grp * 2
                    nc.sync.dma_start(dbg["d_G"][:][:, c0 : c0 + 2], bo[:])

            scT = {}
            shT = {}
            bnT = {}

            def compute_scales(layer, part):
                """part 0: combo col 0 (needs support AR); part 1: cols 1,2 (needs query)."""
                if layer not in scT:
                    scT[layer] = cp.tile([128, 3], F32, tag=f"sc{layer}", name=f"sc{layer}")
                    shT[layer] = cp.tile([128, 3], F32, tag=f"sh{layer}", name=f"sh{layer}")
                    bnT[layer] = cp.tile([128, 6], F32, tag=f"bnT{layer}", name=f"bnT{layer}")
                T = bnT[layer]
                sup = bout[(layer, 0)]
                if part == 0:
                    nc.sync.dma_start(T[0:64, 0:2], sup[:])
                    nc.sync.dma_start(T[64:128, 0:2], sup[:])
                    cs, ce = 0, 1
                else:
                    qry = bout[(layer, 1)]
                    nc.sync.dma_start(T[0:64, 2:4], sup[:])
                    nc.sync.dma_start(T[64:128, 2:4], qry[:])
                    nc.sync.dma_start(T[0:64, 4:6], qry[:])
                    nc.sync.dma_start(T[64:128, 4:6], qry[:])
                    cs, ce = 1, 3
                n = ce - cs
                ic = invct[:, (layer - 1) * 3 + cs : (layer - 1) * 3 + ce]
                sums = T[:, 2 * cs : 2 * ce : 2]
                sss = T[:, 2 * cs + 1 : 2 * ce : 2]
                m = cp.tile([128, 3], F32, tag="bn_m", name="bn_m")[:, 0:n]
                v = cp.tile([128, 3], F32, tag="bn_v", name="bn_v")[:, 0:n]
                msq = cp.tile([128, 3], F32, tag="bn_msq", name="bn_msq")[:, 0:n]
                nc.vector.tensor_tensor(m, sums, ic, ALU.mult)
                nc.vector.tensor_tensor(v, sss, ic, ALU.mult)
                nc.vector.tensor_tensor(msq, m, m, ALU.mult)
                nc.vector.tensor_tensor(v, v, msq, ALU.subtract)
                nc.vector.tensor_scalar(v, v, EPS, None, ALU.add)
                nc.scalar.sqrt(v, v)
                nc.vector.reciprocal(v, v)
                g_b = bnpt[:, (layer - 1) * 2 : (layer - 1) * 2 + 1].broadcast_to((128, n))
                b_b = bnpt[:, (layer - 1) * 2 + 1 : (layer - 1) * 2 + 2].broadcast_to((128, n))
                sc = scT[layer][:, cs:ce]
                sh = shT[layer][:, cs:ce]
                nc.vector.tensor_tensor(sc, v, g_b, ALU.mult)
                nc.vector.tensor_tensor(msq, m, sc, ALU.mult)
                nc.vector.tensor_tensor(sh, b_b, msq, ALU.subtract)

            def combo_col(p):
                return 0 if p < 2 else (1 if p == 2 else 2)

            def relu_apply(layer, view, c, lane):
                """view: AP to normalize+relu in place. c: combo col. lane: 's'|'v'|'g'."""
                sc = scT[layer][:, c : c + 1]
                sh = shT[layer][:, c : c + 1]
                if lane == "s":
                    nc.scalar.activation(view, view, AF.Relu, bias=sh, scale=sc)
                else:
                    eng = nc.vector if lane == "v" else nc.gpsimd
                    eng.tensor_scalar(view, view, sc, sh, ALU.mult, ALU.add)
                    eng.tensor_scalar(view, view, 0.0, None, ALU.max)

            def stats_pair(layer, p, view, nelem):
                """sum via vector reduce; sumsq via scalar Square w/ accum (baseline style)."""
                st = stats[layer]
                nc.vector.tensor_reduce(st[:, 2 * p : 2 * p + 1], view, axis=AX.X, op=ALU.add)
                sq = sq_scr[:, :nelem]
                nc.scalar.activation(sq, view, AF.Square,
                                     accum_out=st[:, 2 * p + 1 : 2 * p + 2])

            # ================ conv1 + pool1 ================
            # im2col: 27 partitions per image (p = di*9+dj*3+ch), 2 images -> K=54
            imcs = [cp.tile([54, 3528], BF16, tag=f"imc{i}", name=f"imc{i}") for i in range(4)]
            scrs = cp  # lane-b scratch allocated on the fly via tags

            dma_engines = [nc.sync, nc.scalar, nc.gpsimd]

            def conv1_pair(p):
                for half, (r0, nr) in enumerate([(0, 42), (42, 40)]):
                    imc = imcs[(p * 2 + half) % 4]
                    L = nr * 84
                    for hb in range(2):
                        img = 2 * p + hb
                        iap = imgs[:]
                        for ch in range(3):
                            srcv = bass.AP(
                                tensor=iap.tensor,
                                offset=iap.offset + img * IMGSTRIDE + ch * 7056 + r0 * 84,
                                ap=[[84, 3], [1, 3], [1, L]],
                            )
                            eng = dma_engines[(p * 4 + half * 2 + hb + ch) % 3]
                            eng.dma_start(imc[hb * 27 + ch * 9 : hb * 27 + ch * 9 + 9, 0:L], srcv)
                    nrow_left = nr
                    c0 = 0
                    ci = 0
                    while nrow_left > 0:
                        cr = min(6, nrow_left)
                        n = cr * 84
                        acc = psum.tile([128, 512], F32, tag="ps", name="ps")
                        nc.tensor.matmul(
                            acc[:, :n], w1t[0:54, :], imc[0:54, c0 : c0 + n],
                            start=True, stop=True, skip_group_check=True,
                        )
                        prow = (r0 + c0 // 84) // 2
                        out_ap = p1[p][:, prow * 41 : (prow + cr // 2) * 41].rearrange(
                            "p (a b) -> p a b", a=cr // 2)
                        nc.vector.tensor_reduce(
                            out_ap,
                            _apv(acc[:, :n], 0, [[168, cr // 2], [2, 41], [84, 2], [1, 2]]),
                            axis=AX.XY, op=ALU.max)
                        c0 += n
                        nrow_left -= cr
                        ci += 1
                stats_pair(1, p, p1[p][:, :1681], 1681)

            for p in range(3):
                conv1_pair(p)
                if p == 0:
                    # warmup collective: absorbs cross-core launch skew
                    wub_in = dram.tile([1, 8], F32, name="wubi")
                    wub_out = dram.tile([1, 8], F32, name="wubo")
                    wu_s = pp.tile([1, 16], F32)
                    nc.sync.dma_start(wu_s[:, 0:8], zbf[:][0:1, 0:16].bitcast(F32))
                    nc.sync.dma_start(wub_in[:], wu_s[:, 0:8])
                    nc.gpsimd.collective_compute(
                        "AllReduce", ALU.add, ins=[wub_in.opt()], outs=[wub_out.opt()],
                        replica_groups=[list(range(8))],
                    )
            launch_group_allreduce(1, 0)
            for p in range(3, NPAIR):
                conv1_pair(p)
            launch_group_allreduce(1, 1)

            # tail-phase weights, loaded up-front (overlap with conv phases)
            gwsA_t = pp.tile([64, 256], BF16)
            gwqB_t = pp.tile([64, 256], BF16)
            nc.scalar.dma_start(gwsA_t[:], gwsA[:])
            nc.scalar.dma_start(gwqB_t[:], gwqB[:])
            abase_t = pp.tile([128, 18], F32)
            qbase_t = pp.tile([128, 18], F32)
            nc.scalar.dma_start(abase_t[:], abase[:])
            nc.scalar.dma_start(qbase_t[:], qbase[:])
            gwt = {}
            gbt = {}
            for i, (w, b) in enumerate([(gw2t, gb2t), (gw3t, gb3t), (gw4t, gb4t)]):
                gwt[i] = pp.tile([128, 512], BF16, tag=f"gwt{i}", name=f"gwt{i}")
                nc.scalar.dma_start(gwt[i][:], w[:])
                gbt[i] = pp.tile([128, 2], F32, tag=f"gbt{i}", name=f"gbt{i}")
                nc.scalar.dma_start(gbt[i][:], b[:])
            fw1 = pp.tile([128, 512], BF16)
            fw2 = pp.tile([128, 512], BF16)
            fw3 = pp.tile([128, 256], BF16)
            fw4 = pp.tile([128, 64], BF16)
            nc.scalar.dma_start(fw1[:], fw1t[:])
            nc.scalar.dma_start(fw2[:], fw2t[:])
            nc.scalar.dma_start(fw3[:], fw3t[:])
            nc.scalar.dma_start(fw4[:], fw4t[:])
            fb1 = pp.tile([128, 2], F32)
            fb2 = pp.tile([128, 2], F32)
            fb3 = pp.tile([128, 1], F32)
            misct = pp.tile([128, 2], F32R)
            nc.scalar.dma_start(fb1[:], fb1t[:])
            nc.scalar.dma_start(fb2[:], fb2t[:])
            nc.scalar.dma_start(fb3[:], fb3t[:])
            nc.scalar.dma_start(misct[:], miscb[:].bitcast(F32R))

            # ================ conv2 + pool2 ================
            CHUNKS2 = [(0, 12), (12, 12), (24, 12), (36, 4)]

            def conv2_pair(p):
                for ci, (r0, nr) in enumerate(CHUNKS2):
                    n = nr * 41
                    acc = psum.tile([128, 512], F32, tag="ps", name="ps")
                    for t in range(9):
                        di, dj = t // 3, t % 3
                        off = di * 41 + dj
                        nc.tensor.matmul(
                            acc[:, :n], wblk[2][:, t * 128 : (t + 1) * 128],
                            p1[p][:, r0 * 41 + off : r0 * 41 + off + n],
                            start=(t == 0), stop=(t == 8), skip_group_check=True,
                        )
                    nu = 12 if nr == 12 else 2
                    prow = r0 // 2
                    out_ap = p2[p][:, prow * 19 : (prow + nu // 2) * 19].rearrange(
                        "p (a b) -> p a b", a=nu // 2)
                    nc.vector.tensor_reduce(
                        out_ap,
                        _apv(acc[:, :n], 0, [[82, nu // 2], [2, 19], [41, 2], [1, 2]]),
                        axis=AX.XY, op=ALU.max)
                stats_pair(2, p, p2[p][:, :361], 361)

            RELU_LANES = ["s", "v", "s", "v", "s", "v", "s", "v", "s", "v"]

            compute_scales(1, 0)
            for p in range(2):
                relu_apply(1, p1[p][:, :1681], 0, "s" if p == 0 else "v")
                if debug:
                    nc.sync.dma_start(dbg["d_p1n"][:][:, p * 1681 : (p + 1) * 1681], p1[p][:, :1681])
                conv2_pair(p)
            compute_scales(1, 1)
            for p in range(2, NPAIR):
                relu_apply(1, p1[p][:, :1681], combo_col(p), RELU_LANES[p])
                if debug:
                    nc.sync.dma_start(dbg["d_p1n"][:][:, p * 1681 : (p + 1) * 1681], p1[p][:, :1681])
            conv2_pair(2)
            launch_group_allreduce(2, 0)
            for p in range(3, NPAIR):
                conv2_pair(p)
            launch_group_allreduce(2, 1)

            # ================ conv3 (no pool) ================
            def conv34_pair(layer, p, src, dst, nvalid):
                if layer == 3:
                    nc.sync.dma_start(dst[:, 0:328], zbf[:][:, 0:328])
                acc = psum.tile([128, 512], F32, tag="ps", name="ps")
                for t in range(9):
                    di, dj = t // 3, t % 3
                    off = di * 19 + dj
                    n = 324 if layer == 3 else 288
                    nc.tensor.matmul(
                        acc[:, :n], wblk[layer][:, t * 128 : (t + 1) * 128],
                        src[:, off : off + n],
                        start=(t == 0), stop=(t == 8), skip_group_check=True,
                    )
                w = nvalid  # 17 or 15
                vps = acc[:, : w * 19].rearrange("p (a b) -> p a b", a=w)[:, :, 0:w]
                vdst = dst[:, : w * 19].rearrange("p (a b) -> p a b", a=w)[:, :, 0:w]
                st = stats[layer]
                nc.scalar.activation(vdst, vps, AF.Copy, accum_out=st[:, 2 * p : 2 * p + 1])
                sq = sq_scr[:, : w * w].rearrange("p (a b) -> p a b", a=w)
                nc.scalar.activation(sq, vdst, AF.Square,
                                     accum_out=st[:, 2 * p + 1 : 2 * p + 2])

            compute_scales(2, 0)
            for p in range(2):
                relu_apply(2, p2[p][:, :361], 0, "s" if p == 0 else "v")
                if debug:
                    nc.sync.dma_start(dbg["d_p2n"][:][:, p * 361 : (p + 1) * 361], p2[p][:, :361])
                conv34_pair(3, p, p2[p][:], l3[p][:], 17)
            compute_scales(2, 1)
            for p in range(2, NPAIR):
                relu_apply(2, p2[p][:, :361], combo_col(p), RELU_LANES[p])
                if debug:
                    nc.sync.dma_start(dbg["d_p2n"][:][:, p * 361 : (p + 1) * 361], p2[p][:, :361])
            conv34_pair(3, 2, p2[2][:], l3[2][:], 17)
            launch_group_allreduce(3, 0)
            for p in range(3, NPAIR):
                conv34_pair(3, p, p2[p][:], l3[p][:], 17)
            launch_group_allreduce(3, 1)

            # ================ conv4 (no pool) ================
            def vview(t, w):
                return t[:, : w * 19].rearrange("p (a b) -> p a b", a=w)[:, :, 0:w]

            compute_scales(3, 0)
            for p in range(2):
                relu_apply(3, vview(l3[p][:], 17), 0, "s" if p == 0 else "v")
                if debug:
                    nc.sync.dma_start(dbg["d_l3n"][:][:, p * 323 : (p + 1) * 323], l3[p][:, :323])
                conv34_pair(4, p, l3[p][:], l4[p][:], 15)
            compute_scales(3, 1)
            for p in range(2, NPAIR):
                relu_apply(3, vview(l3[p][:], 17), combo_col(p), RELU_LANES[p])
                if debug:
                    nc.sync.dma_start(dbg["d_l3n"][:][:, p * 323 : (p + 1) * 323], l3[p][:, :323])
            conv34_pair(4, 2, l3[2][:], l4[2][:], 15)
            launch_group_allreduce(4, 0)
            for p in range(3, NPAIR):
                conv34_pair(4, p, l3[p][:], l4[p][:], 15)
            launch_group_allreduce(4, 1)

            # ================ relu4 + avgpool ================
            fall = cp.tile([128, 90], F32, tag="fall")

            def pool4(p):
                inv = _apv(l4[p][:], 0, [[95, 3], [5, 3], [19, 5], [1, 5]])
                nc.vector.tensor_reduce(fall[:, p * 9 : (p + 1) * 9], inv, axis=AX.XY, op=ALU.add)

            compute_scales(4, 0)
            for p in range(2):
                relu_apply(4, vview(l4[p][:], 15), 0, "s" if p == 0 else "v")
                if debug:
                    nc.sync.dma_start(dbg["d_l4n"][:][:, p * 285 : (p + 1) * 285], l4[p][:, :285])
                pool4(p)
            compute_scales(4, 1)
            for p in range(2, NPAIR):
                relu_apply(4, vview(l4[p][:], 15), combo_col(p), RELU_LANES[p])
                if debug:
                    nc.sync.dma_start(dbg["d_l4n"][:][:, p * 285 : (p + 1) * 285], l4[p][:, :285])
                pool4(p)
            # convert to bf16 and assemble feats (ch, img*9 + p)
            fallb = cp.tile([128, 90], BF16, tag="fallb")
            nc.vector.tensor_copy(fallb[:], fall[:])
            for hb in range(2):
                dstv = _apv(feats[:, hb * 9 : hb * 9 + 9], 0, [[18, 10], [1, 9]])
                srcv = _apv(fallb[hb * 64 : hb * 64 + 64, :], 0, [[9, 10], [1, 9]])
                nc.sync.dma_start(dstv, srcv)
            if debug:
                nc.sync.dma_start(dbg["d_f64"][:], feats[:])

            cpool_cm.__exit__(None, None, None)

            # ================ g-MLP ================
            tpool_cm = tc.tile_pool(name="tailp", bufs=1)
            tp = tpool_cm.__enter__()

            A_f = [tp.tile([128, 45], BF16, tag=f"A_f{k}", name=f"A_f{k}") for k in range(2)]
            B_f = [tp.tile([128, 135], BF16, tag=f"B_f{k}", name=f"B_f{k}") for k in range(2)]
            for mh in range(2):
                accA = psum.tile([128, 48], F32, tag="ps", name="ps")
                nc.tensor.matmul(accA[:], gwsA_t[:, mh * 128 : (mh + 1) * 128],
                                 feats[:, 0:48], start=True, stop=True)
                bav = abase_t[:, mh * 9 : (mh + 1) * 9].unsqueeze(1).broadcast_to((128, 5, 9))
                nc.vector.tensor_tensor(
                    A_f[mh][:].rearrange("p (a b) -> p a b", a=5),
                    accA[:, :45].rearrange("p (a b) -> p a b", a=5), bav, ALU.add)
                accB = psum.tile([128, 136], F32, tag="ps", name="ps")
                nc.tensor.matmul(accB[:], gwqB_t[:, mh * 128 : (mh + 1) * 128],
                                 feats[:, 45:181], start=True, stop=True)
                qbv = qbase_t[:, mh * 9 : (mh + 1) * 9].unsqueeze(1).broadcast_to((128, 15, 9))
                nc.vector.tensor_tensor(
                    B_f[mh][:].rearrange("p (a b) -> p a b", a=15),
                    accB[:, :135].rearrange("p (a b) -> p a b", a=15), qbv, ALU.add)

            NH = 6076  # 45 * 135 + 1 pad
            SPL = 28   # h-build split point (vector: sp<SPL, gpsimd: rest)
            with tc.tile_pool(name="hpool", bufs=4) as hpool:
                h_in = [hpool.tile([128, NH], BF16, tag="h", name="h") for _ in range(2)]
                for kh in range(2):
                    # h[sp*135 + j] = relu(B[j] + A[sp]); add via broadcast TT, relu on scalar
                    bv = _apv(B_f[kh][:], 0, [[0, SPL], [1, 135]])
                    av = _apv(A_f[kh][:], 0, [[1, SPL], [0, 135]])
                    ov = _apv(h_in[kh][:], 0, [[135, SPL], [1, 135]])
                    nc.vector.tensor_tensor(ov, bv, av, ALU.add)
                    bv2 = _apv(B_f[kh][:], 0, [[0, 45 - SPL], [1, 135]])
                    av2 = _apv(A_f[kh][:], SPL, [[1, 45 - SPL], [0, 135]])
                    ov2 = _apv(h_in[kh][:], SPL * 135, [[135, 45 - SPL], [1, 135]])
                    nc.vector.tensor_tensor(ov2, bv2, av2, ALU.add)
                    nc.scalar.activation(h_in[kh][:, 0 : SPL * 135], h_in[kh][:, 0 : SPL * 135], AF.Relu)
                    nc.scalar.activation(h_in[kh][:, SPL * 135 : 6075], h_in[kh][:, SPL * 135 : 6075], AF.Relu)
                    nc.sync.dma_start(h_in[kh][:, 6075:6076], zbf[:][:, 0:1])

                chunks = [(i * 512, 512) for i in range(11)] + [(5632, 444)]
                for li in range(3):
                    h_out = [hpool.tile([128, NH], BF16, tag="h", name="h") for _ in range(2)]
                    for mh in range(2):
                        for ci, (c0, n) in enumerate(chunks):
                            acc = psum.tile([128, 512], F32, tag="ps", name="ps")
                            nc.tensor.matmul(acc[:, :n], gwt[li][:, mh * 128 : mh * 128 + 128],
                                             h_in[0][:, c0 : c0 + n], start=True, stop=False)
                            nc.tensor.matmul(acc[:, :n], gwt[li][:, 256 + mh * 128 : 256 + mh * 128 + 128],
                                             h_in[1][:, c0 : c0 + n], start=False, stop=True)
                            dst = h_out[mh][:, c0 : c0 + n]
                            if (ci + mh) % 2 == 0:
                                nc.scalar.activation(dst, acc[:, :n], AF.Relu,
                                                     bias=gbt[li][:, mh : mh + 1])
                            else:
                                nc.vector.tensor_scalar(dst, acc[:, :n], gbt[li][:, mh : mh + 1],
                                                        0.0, ALU.add, ALU.max)
                        if li == 2:
                            nc.sync.dma_start(h_out[mh][:, 6075:6076], zbf[:][:, 0:1])
                    h_in = h_out

                # x_f[(s,q)] = sum over (p1,p2) of h[(s,p1,q,p2)]
                xff = [tp.tile([128, 76], F32, tag=f"xff{k}", name=f"xff{k}") for k in range(2)]
                for kh in range(2):
                    hv = h_in[kh][:]
                    for s in range(5):
                        inv = _apv(hv, s * 1215, [[9, 15], [135, 9], [1, 9]])
                        nc.vector.tensor_reduce(
                            xff[kh][:, s * 15 : (s + 1) * 15], inv, axis=AX.XY, op=ALU.add)
                if debug:
                    for kh in range(2):
                        nc.sync.dma_start(dbg["d_xf"][:][:, kh * 76 : (kh + 1) * 76], xff[kh][:])

            xfb = [tp.tile([128, 76], BF16, tag=f"xfb{k}", name=f"xfb{k}") for k in range(2)]
            for kh in range(2):
                nc.vector.tensor_copy(xfb[kh][:, 0:75], xff[kh][:, 0:75])
                nc.sync.dma_start(xfb[kh][:, 75:76], zbf[:][:, 0:1])

            # ================ f-MLP + score + loss ================
            y_in = xfb
            for li, (w, bias) in enumerate([(fw1, fb1), (fw2, fb2)]):
                y_out = [tp.tile([128, 76], BF16, tag=f"y{li}_{k}", name=f"y{li}_{k}") for k in range(2)]
                for mh in range(2):
                    acc = psum.tile([128, 76], F32, tag="ps", name="ps")
                    nc.tensor.matmul(acc[:], w[:, mh * 128 : mh * 128 + 128],
                                     y_in[0][:], start=True, stop=False)
                    nc.tensor.matmul(acc[:], w[:, 256 + mh * 128 : 256 + mh * 128 + 128],
                                     y_in[1][:], start=False, stop=True)
                    nc.scalar.activation(y_out[mh][:], acc[:], AF.Relu,
                                         bias=bias[:, mh : mh + 1])
                y_in = y_out
            y3 = tp.tile([128, 76], BF16, tag="y3")
            acc = psum.tile([128, 76], F32, tag="ps", name="ps")
            nc.tensor.matmul(acc[:], fw3[:, 0:128], y_in[0][:], start=True, stop=False)
            nc.tensor.matmul(acc[:], fw3[:, 128:256], y_in[1][:], start=False, stop=True)
            nc.scalar.activation(y3[:], acc[:], AF.Relu, bias=fb3[:, 0:1])
            acc4 = psum.tile([64, 76], F32, tag="ps", name="ps")
            nc.tensor.matmul(acc4[:], fw4[:], y3[:], start=True, stop=True)
            osq = tp.tile([64, 76], F32R, tag="osq")
            nc.scalar.activation(osq[:], acc4[:], AF.Square,
                                 bias=misct[0:64, 0:1].bitcast(F32))
            acc_sc = psum.tile([1, 76], F32, tag="ps", name="ps")
            nc.tensor.matmul(acc_sc[:], misct[0:64, 1:2], osq[:], start=True, stop=True)
            sc2t = tp.tile([1, 76], F32, tag="sc2")
            nc.vector.tensor_copy(sc2t[:], acc_sc[:])
            nc.sync.dma_start(loss_out[:], sc2t[:, :75])
            tpool_cm.__exit__(None, None, None)

    nc.compile()
    return nc


# ---------------------------------------------------------------- entry point
_CACHE = {}


def finish_loss(results, inputs):
    """Host epilogue: squash + margin loss from per-core score^2 (75 flops/core)."""
    sy = np.asarray(inputs["support_y"])
    qy = np.asarray(inputs["query_y"])
    total = np.float32(0.0)
    for b in range(B):
        sc2 = np.asarray(results[b]["loss"][0], np.float32)  # (75,) col = s*15+q
        score = np.sqrt(np.maximum(sc2, 0.0)).reshape(5, 15).T  # (q, s)
        n = np.sqrt((score * score).sum(1, keepdims=True))
        score = score / n * (n * n / (1.0 + n * n))
        ap = sy[b][None, :] == qy[b][:, None]
        sap = np.sum(np.where(ap, score, 0.0), axis=1, keepdims=True)
        total += np.float32(np.sum(np.maximum(score - sap + 0.2, 0.0) * (~ap)))
    return np.array(total, dtype=np.float32)


def kernel(**inputs) -> np.ndarray:
    if "nc" not in _CACHE:
        _CACHE["nc"] = build_kernel(debug=False)
    nc = _CACHE["nc"]
    packed = _pack_weights(inputs)
    in_maps = []
    for b in range(B):
        m = dict(packed)
        m.update(_per_core_inputs(inputs, b))
        in_maps.append(m)
    res = run_bass_kernel_spmd(nc, in_maps, core_ids=list(range(8)))
    return finish_loss(res.results, inputs)
